# revision 7
# baseline (speedup 1.0000x reference)
"""Trainium2 Bass kernel for nn_BondPredictor (gnn_message_passing) — v3.

Computes, for each batch b:
    A      = hidden_states[b][clip(atom_indices[b])]          # [256, 512] gather
    pair   = concat(A[i]+A[j], |A[i]-A[j]|)                   # [256,256,1024]
    h      = gelu(pair @ W1 + b1)                             # [256,256,512]
    logits = h @ W2 + b2  -> [7, 256, 256], diagonal = -10000

Sharding: 8 cores = 2 batches x 4 row-blocks of 64 rows; atom axis rolled by
-64*(c%4) per core (pure SPMD); host un-rolls on unshard. Pair symmetry:
each row computes cyclic offsets (j-i) mod 256 in [0,128]; the host mirrors
offsets 129..255 from the transpose.

v3 engine plan (vs the bf16 v2 baseline at ~130us):
- Gather via indirect DMA + DMA transposes (no one-hot matmuls).
- |d| = 2 relu(d) - d; relu features quantized to fp8e4 by DVE dual-op
  tensor_scalar (sub,max) at the 2x SBUF rate; the per-pair contraction runs
  as fp8 DoubleRow matmuls (2x PE), first-layer weights prescaled x128 on
  the host (x64 fp8-range scale, x2 from the relu identity); the gelu
  activation applies scale=1/64 to descale.
- Rank-one injections ws*(P-Q)[j] and ws*(P+Q+b1)[i] enter PSUM through
  identity-weight DoubleRow matmuls whose moving operand is a 4-D strided
  AP (overlapping column windows for the j-term; inner stride-0 broadcast
  for the i-term). Zero per-quad DVE cost for the rank-one terms.
- P/Q phase also fp8 DoubleRow with host-prepped ws*(W1a-+W1b) weights.
- Second layer bf16; outputs of 3 quads packed into one PSUM bank at
  partition offsets 0/32/64 so the b2-add drain is one DVE op per 3 quads.
- Diagonal fill and the symmetry mirror happen on the host.
"""

import sys

sys.path.insert(0, "/opt/trn_rl_repo")

import numpy as np
import ml_dtypes

F8 = ml_dtypes.float8_e4m3
BF = ml_dtypes.bfloat16

B, T, D, N, C = 2, 1024, 512, 256, 7
NCORES = 8
RB = 4                # row-blocks per batch
NL = N // RB          # 64 rows per core
QR = 4                # rows per quad
NQ = NL // QR         # 16 quads
KC = D // 128         # 4 chunks of the 512-dim contraction
TW = 128              # cols per row: cyclic offsets (j-i) in [0,127]
MASK_FILL = -10000.0
WS = 64.0             # fp8 weight prescale

POOL_CHUNKS = 2       # absq feature-chunks whose subtract runs on GpSimd

_CACHE = {}


def _build(reps=1, sim_compat=False):
    import concourse.bass as bass
    import concourse.bacc as bacc
    import concourse.tile as tile
    from concourse import mybir

    f32 = mybir.dt.float32
    bf16 = mybir.dt.bfloat16
    fp8 = mybir.dt.float8e4
    i32 = mybir.dt.int32
    Alu = mybir.AluOpType
    Act = mybir.ActivationFunctionType
    DR = mybir.MatmulPerfMode.DoubleRow
    ACTF = Act.Relu if sim_compat else Act.Gelu

    nc = bacc.Bacc("TRN2", target_bir_lowering=False, debug=False)

    h_d = nc.dram_tensor("h", [T, D], bf16, kind="ExternalInput")
    idx_d = nc.dram_tensor("idx", [128, 2], i32, kind="ExternalInput")
    w1b8_d = nc.dram_tensor("w1b8", [D, D], fp8, kind="ExternalInput")
    wpm8_d = nc.dram_tensor("wpm8", [D, D], fp8, kind="ExternalInput")
    wpp8_d = nc.dram_tensor("wpp8", [D, D], fp8, kind="ExternalInput")
    w2b_d = nc.dram_tensor("w2b", [128, KC * C], bf16, kind="ExternalInput")
    w28_d = nc.dram_tensor("w28", [128, 64], fp8, kind="ExternalInput")
    b1s_d = nc.dram_tensor("b1s", [128, KC], f32, kind="ExternalInput")
    b2r_d = nc.dram_tensor("b2r", [128, 1], f32, kind="ExternalInput")
    id8_d = nc.dram_tensor("id8", [128, 256], fp8, kind="ExternalInput")
    out1_d = nc.dram_tensor("out1", [C, NQ * 512], f32, kind="ExternalOutput")
    out2_d = nc.dram_tensor("out2", [C, NL], f32, kind="ExternalOutput")

    with tile.TileContext(nc) as tc:
        from contextlib import ExitStack

        with ExitStack() as ctx:
            const = ctx.enter_context(tc.tile_pool(name="const", bufs=1))
            gpool = ctx.enter_context(tc.tile_pool(name="g", bufs=2))
            work = ctx.enter_context(tc.tile_pool(name="work", bufs=3))
            opool = ctx.enter_context(tc.tile_pool(name="o", bufs=2))
            # PSUM: ph 2 bufs x 2 banks + po_c 2 x 1 + po_w 2 x 1 = 8 banks
            ph = ctx.enter_context(
                tc.tile_pool(name="ph", bufs=3, space=bass.MemorySpace.PSUM)
            )
            po_c = ctx.enter_context(
                tc.tile_pool(name="po_c", bufs=1, space=bass.MemorySpace.PSUM)
            )
            po_w = ctx.enter_context(
                tc.tile_pool(name="po_w", bufs=1, space=bass.MemorySpace.PSUM)
            )

            # ---- one-time constants ----
            id8 = const.tile([128, 256], fp8, tag="id8")
            nc.sync.dma_start(id8[:], id8_d.ap())
            id8_3d = id8[:].rearrange("p (s m) -> p s m", s=2)
            idb = const.tile([128, 128], bf16, tag="idb")
            nc.vector.tensor_copy(idb[:], id8[:, 0:128])

            def slot(ap_, sl):
                dims = [list(d) for d in ap_.ap]
                return bass.AP(
                    tensor=ap_.tensor,
                    offset=ap_.offset + sl * dims[1][0],
                    ap=[dims[0]] + dims[2:],
                )

            def mm_dr(out, lhsT, rhs, start, stop):
                """DoubleRow matmul; in sim_compat, lower to interp-friendly
                non-DR matmuls (slot loop) with identical operands/deps."""
                if not sim_compat:
                    nc.tensor.matmul(out, lhsT, rhs, start=start, stop=stop,
                                     perf_mode=DR)
                    return
                for sl in range(2):
                    nc.tensor.matmul(out, slot(lhsT, sl), slot(rhs, sl),
                                     start=(start and sl == 0),
                                     stop=(stop and sl == 1))
            w1b8 = const.tile([128, KC * 512], fp8, tag="w1b8")
            wpm8 = const.tile([128, KC * 512], fp8, tag="wpm8")
            wpp8 = const.tile([128, KC * 512], fp8, tag="wpp8")
            for m in range(KC):
                sl = slice(512 * m, 512 * (m + 1))
                rows = slice(128 * m, 128 * (m + 1))
                nc.sync.dma_start(w1b8[:, sl], w1b8_d.ap()[rows, :])
                nc.sync.dma_start(wpm8[:, sl], wpm8_d.ap()[rows, :])
                nc.sync.dma_start(wpp8[:, sl], wpp8_d.ap()[rows, :])
            w2sb = const.tile([128, KC * C], bf16, tag="w2sb")
            nc.sync.dma_start(w2sb[:], w2b_d.ap())
            w28sb = const.tile([128, 64], fp8, tag="w28sb")
            nc.sync.dma_start(w28sb[:], w28_d.ap())
            b1s = const.tile([128, KC], f32, tag="b1s")
            nc.sync.dma_start(b1s[:], b1s_d.ap())
            b2r = const.tile([128, 1], f32, tag="b2r")
            nc.sync.dma_start(b2r[:], b2r_d.ap())

            def wtile(t, m):
                return t[:, 512 * m : 512 * (m + 1)].rearrange(
                    "p (k f) -> p k f", k=KC
                )

            def prep():
                # ---- gather A = h[idx] (atom-major), transpose to f-major ----
                idx_sb = gpool.tile([128, 2], i32, tag="idx_sb")
                nc.sync.dma_start(idx_sb[:], idx_d.ap())
                ga = []
                for t_ in range(2):
                    g = gpool.tile([128, D], bf16, tag=f"ga{t_}")
                    nc.gpsimd.indirect_dma_start(
                        out=g[:], out_offset=None, in_=h_d.ap(),
                        in_offset=bass.IndirectOffsetOnAxis(
                            ap=idx_sb[:, t_ : t_ + 1], axis=0
                        ),
                    )
                    ga.append(g)
                at = gpool.tile([128, KC, N], bf16, tag="at")
                for t_ in range(2):
                    for k in range(KC):
                        eng = nc.sync if (k % 2 == 0) else nc.scalar
                        eng.dma_start_transpose(
                            at[:, k, 128 * t_ : 128 * (t_ + 1)],
                            ga[t_][:, 128 * k : 128 * (k + 1)],
                        )
                at8 = gpool.tile([128, KC, N], fp8, tag="at8")
                nc.vector.tensor_copy(
                    at8[:].rearrange("p k a -> p (k a)"),
                    at[:].rearrange("p k a -> p (k a)"),
                )
                ai32 = gpool.tile([128, KC, NL], f32, tag="ai32")
                nc.vector.tensor_copy(
                    ai32[:].rearrange("p k a -> p (k a)"), at[:, :, 0:NL]
                )

                # ---- P/Q phase: p2 = ws(P-Q) all atoms, pqb8 = ws(P+Q+b1)[0:64]
                p2, pqb8 = [], []
                for m in range(KC):
                    ps_c = po_c.tile([128, 512], f32, tag="po_c")
                    for kk in range(2):
                        mm_dr(
                            ps_c[:, 0:N],
                            wtile(wpm8, m)[:, 2 * kk : 2 * kk + 2, :],
                            at8[:, 2 * kk : 2 * kk + 2, :],
                            start=(kk == 0), stop=(kk == 1),
                        )
                    p = gpool.tile([128, N], fp8, tag=f"p2_{m}")
                    nc.vector.tensor_copy(p[:], ps_c[:, 0:N])
                    p2.append(p)
                for m in range(KC):
                    ps_q = po_c.tile([128, 512], f32, tag="po_c")
                    for kk in range(2):
                        mm_dr(
                            ps_q[:, 0:NL],
                            wtile(wpp8, m)[:, 2 * kk : 2 * kk + 2, :],
                            at8[:, 2 * kk : 2 * kk + 2, 0:NL],
                            start=(kk == 0), stop=(kk == 1),
                        )
                    pq = gpool.tile([128, 72], fp8, tag=f"pqb8_{m}")
                    nc.vector.tensor_scalar(
                        pq[:, 0:NL], ps_q[:, 0:NL], b1s[:, m : m + 1], None,
                        op0=Alu.add,
                    )
                    nc.vector.memset(pq[:, NL:72], 0.0)
                    pqb8.append(pq)
                return at, ai32, p2, pqb8

            def main(st):
                at, ai32, p2, pqb8 = st

                def p2win(m, q0):
                    base = p2[m][:, 0:1]
                    return bass.AP(
                        tensor=base.tensor, offset=base.offset + q0,
                        ap=[list(base.ap[0]), [16, 2], [1, QR], [1, TW]],
                    )

                def pqbwin(m, q0):
                    base = pqb8[m][:, 0:1]
                    return bass.AP(
                        tensor=base.tensor, offset=base.offset + q0,
                        ap=[list(base.ap[0]), [4, 2], [1, QR], [0, TW]],
                    )

                # ---- antipodal pass: pairs (i, i+128), i in 0..63 ----
                absA = work.tile([128, KC, NL], fp8, tag="absA")
                dA = work.tile([128, KC, NL], bf16, tag="dA")
                for k in range(KC):
                    nc.vector.tensor_tensor(
                        dA[:, k, :], at[:, k, TW : TW + NL], at[:, k, 0:NL],
                        op=Alu.subtract,
                    )
                    nc.vector.tensor_scalar(
                        absA[:, k, :], dA[:, k, :], 0.0, None, op0=Alu.max
                    )
                hhA = work.tile([128, KC * NL], bf16, tag="hhA")
                for m in range(KC):
                    ps_a = po_w.tile([128, 512], f32, tag="po_w")
                    bank = ps_a[:, 0:NL]
                    jsrc = p2[m][:, 0:1]
                    j_ap = bass.AP(
                        tensor=jsrc.tensor, offset=jsrc.offset + TW,
                        ap=[list(jsrc.ap[0]), [64, 2], [1, NL]],
                    )
                    mm_dr(bank, id8_3d, j_ap, start=True, stop=False)
                    isrc = pqb8[m][:, 0:1]
                    i_ap = bass.AP(
                        tensor=isrc.tensor, offset=isrc.offset,
                        ap=[list(isrc.ap[0]), [4, 2], [1, NL]],
                    )
                    mm_dr(bank, id8_3d, i_ap, start=False, stop=False)
                    for kk in range(2):
                        mm_dr(
                            bank,
                            wtile(w1b8, m)[:, 2 * kk : 2 * kk + 2, :],
                            absA[:, 2 * kk : 2 * kk + 2, :],
                            start=False, stop=(kk == 1),
                        )
                    nc.scalar.activation(
                        hhA[:, NL * m : NL * (m + 1)], bank,
                        ACTF, scale=1.0 / WS,
                    )
                ps_o = po_w.tile([128, 512], f32, tag="po_w")
                for k in range(KC):
                    nc.tensor.matmul(
                        ps_o[0:C, 0:NL],
                        w2sb[:, C * k : C * (k + 1)],
                        hhA[:, NL * k : NL * (k + 1)],
                        start=(k == 0), stop=(k == KC - 1),
                    )
                tmpA = opool.tile([C, NL], f32, tag="tmpA")
                nc.vector.tensor_scalar(
                    tmpA[:], ps_o[0:C, 0:NL], b2r[0:C, :], None, op0=Alu.add
                )
                nc.sync.dma_start(out2_d.ap(), tmpA[:])

                # ---- main loop over row-quads ----
                tmp_all = [None]
                for q in range(NQ):
                    if q == 0:
                        tmp_all[0] = opool.tile([C, NQ * 512], f32, tag="tmp_all", name="tmp_all")
                    i0 = QR * q
                    absq = work.tile([128, KC, QR * TW], fp8, tag="absq")
                    dsub = work.tile(
                        [128, max(POOL_CHUNKS, 1), QR * TW], bf16, tag="dsub"
                    )
                    for k in range(KC):
                        if k < POOL_CHUNKS:
                            base = at[:, k, 0:1]
                            j_ap = bass.AP(
                                tensor=base.tensor, offset=base.offset + i0,
                                ap=[list(base.ap[0]), [1, QR], [1, TW]],
                            )
                            i_ap = bass.AP(
                                tensor=base.tensor, offset=base.offset + i0,
                                ap=[list(base.ap[0]), [1, QR], [0, TW]],
                            )
                            nc.gpsimd.tensor_tensor(
                                dsub[:, k, :], j_ap, i_ap, op=Alu.subtract
                            )
                            nc.vector.tensor_scalar(
                                absq[:, k, :], dsub[:, k, :], 0.0, None,
                                op0=Alu.max,
                            )
                        else:
                            for r in range(QR):
                                i = i0 + r
                                nc.vector.tensor_scalar(
                                    absq[:, k, TW * r : TW * (r + 1)],
                                    at[:, k, i : i + TW],
                                    ai32[:, k, i : i + 1],
                                    0.0, op0=Alu.subtract, op1=Alu.max,
                                )

                    hh = work.tile([128, KC * 512], fp8, tag="hh")
                    for mm in range(2):
                        ps_h = ph.tile([128, 1024], f32, tag="ph")
                        for mi, m in enumerate((2 * mm, 2 * mm + 1)):
                            bank = ps_h[:, 512 * mi : 512 * (mi + 1)]
                            mm_dr(bank, id8_3d, p2win(m, i0),
                                  start=True, stop=False)
                            mm_dr(bank, id8_3d, pqbwin(m, i0),
                                  start=False, stop=False)
                            for kk in range(2):
                                mm_dr(
                                    bank,
                                    wtile(w1b8, m)[:, 2 * kk : 2 * kk + 2, :],
                                    absq[:, 2 * kk : 2 * kk + 2, :],
                                    start=False, stop=(kk == 1),
                                )
                        nc.scalar.activation(
                            hh[:, 1024 * mm : 1024 * (mm + 1)], ps_h[:],
                            ACTF, scale=1.0 / WS,
                        )

                    psq2 = po_w.tile([128, 512], f32, tag="po_w", name="psq2")
                    for kk in range(2):
                        mm_dr(
                            psq2[0:C, :],
                            w28sb[:, 32 * kk : 32 * (kk + 1)].rearrange(
                                "p (k c) -> p k c", k=2
                            )[:, :, 0:C],
                            hh[:, 1024 * kk : 1024 * (kk + 1)].rearrange(
                                "p (k n) -> p k n", k=2
                            ),
                            start=(kk == 0), stop=(kk == 1),
                        )
                    dst = tmp_all[0][:, 512 * q : 512 * (q + 1)]
                    if q % 8 < 5:
                        nc.vector.tensor_scalar(
                            dst, psq2[0:C, :], 1.0 / WS, b2r[0:C, :],
                            op0=Alu.mult, op1=Alu.add,
                        )
                    else:
                        nc.scalar.activation(
                            dst, psq2[0:C, :], Act.Identity,
                            bias=b2r[0:C, :], scale=1.0 / WS,
                        )
                    if q == NQ - 1:
                        nc.sync.dma_start(out1_d.ap(), tmp_all[0][:])


            st = prep()
            for r_ in range(reps):
                nxt = prep() if r_ + 1 < reps else None
                main(st)
                st = nxt

    nc.compile()
    return nc


def _get(reps=1, sim_compat=False):
    key = (reps, sim_compat)
    if key not in _CACHE:
        _CACHE[key] = _build(reps, sim_compat)
    return _CACHE[key]


def _prep_weights(W1, b1, W2, b2):
    """Host-side weight packing. Device tile layout per m-block (rows
    128m..128m+127 of the DRAM tensor): tile[p, 128k+f] = w[128k+p, 128m+f],
    i.e. contraction chunk k as weight slot k, output feature f."""
    W1 = np.asarray(W1, np.float32)
    W1a, W1b = W1[0:D], W1[D : 2 * D]

    def pack(w):
        out = np.empty((D, D), np.float32)
        for m in range(KC):
            for k in range(KC):
                out[128 * m : 128 * (m + 1), 128 * k : 128 * (k + 1)] = w[
                    128 * k : 128 * (k + 1), 128 * m : 128 * (m + 1)
                ]
        return out

    clip8 = lambda x: np.clip(x, -240.0, 240.0).astype(F8)
    w1b8 = clip8(pack(2 * WS * W1b))
    wpm8 = clip8(pack(WS * (W1a - W1b)))
    wpp8 = clip8(pack(WS * (W1a + W1b)))
    W2f = np.asarray(W2, np.float32)
    w2b = np.zeros((128, KC * C), np.float32)
    for k in range(KC):
        w2b[:, C * k : C * (k + 1)] = W2f[128 * k : 128 * (k + 1), :]
    w2b = w2b.astype(BF)
    w28 = np.zeros((128, 64), np.float32)
    for k in range(KC):
        w28[:, 16 * k : 16 * k + C] = WS * W2f[128 * k : 128 * (k + 1), :]
    w28 = np.clip(w28, -240.0, 240.0).astype(F8)
    b1f = np.asarray(b1, np.float32)
    b1s = np.zeros((128, KC), np.float32)
    for m in range(KC):
        b1s[:, m] = WS * b1f[128 * m : 128 * (m + 1)]
    b2f = np.asarray(b2, np.float32)
    b2r = np.zeros((128, 1), np.float32)
    for s in range(3):
        b2r[32 * s : 32 * s + C, 0] = b2f
    id8 = np.zeros((128, 256), np.float32)
    id8[:, 0:128] = np.eye(128)
    id8 = id8.astype(F8)
    return w1b8, wpm8, wpp8, w2b, w28, b1s, b2r, id8


def _shard_inputs(hidden_states, W1, b1, W2, b2, atom_indices):
    hs = np.asarray(hidden_states, np.float32)
    idx = np.clip(np.asarray(atom_indices).astype(np.int64), 0, T - 1)
    w1b8, wpm8, wpp8, w2b, w28, b1s, b2r, id8 = _prep_weights(W1, b1, W2, b2)
    in_maps = []
    for c in range(NCORES):
        b = c // RB
        r0 = NL * (c % RB)
        idx_roll = np.roll(idx[b], -r0).astype(np.int32).reshape(2, 128).T
        in_maps.append(
            {
                "h": hs[b].astype(BF),
                "idx": np.ascontiguousarray(idx_roll),
                "w1b8": w1b8, "wpm8": wpm8, "wpp8": wpp8,
                "w2b": w2b, "w28": w28, "b1s": b1s, "b2r": b2r, "id8": id8,
            }
        )
    return in_maps


def _unshard(results, atom_mask):
    full = np.empty((B, C, N, N), np.float32)
    for c in range(NCORES):
        b = c // RB
        r0 = NL * (c % RB)
        o1p = results[c]["out1"]  # [7, 16*512]: quad q at cols 512q
        o2 = results[c]["out2"]  # [7, 64]
        blk = np.empty((C, NL, TW + 1), np.float32)
        blk[:, :, 0:TW] = o1p.reshape(C, NQ, QR, TW).reshape(C, NL, TW)
        
        blk[:, :, TW] = o2
        rows = r0 + np.arange(NL)
        idx_j = (rows[:, None] + np.arange(TW + 1)[None, :]) % N
        np.put_along_axis(
            full[b, :, r0 : r0 + NL, :],
            np.broadcast_to(idx_j[None], (C, NL, TW + 1)),
            blk,
            axis=2,
        )
    offs = (np.arange(N)[None, :] - np.arange(N)[:, None]) % N
    low = offs > TW
    fullT = np.transpose(full, (0, 1, 3, 2))
    full = np.where(low[None, None], fullT, full)
    di = np.arange(N)
    full[:, :, di, di] = MASK_FILL
    mask = np.asarray(atom_mask).astype(bool)
    if not mask.all():
        valid = mask[:, :, None] & mask[:, None, :]
        valid &= ~np.eye(N, dtype=bool)[None]
        full = np.where(valid[:, None, :, :], full, np.float32(MASK_FILL))
    return full


def kernel(hidden_states, W1, b1, W2, b2, atom_indices, atom_mask):
    from concourse.bass_utils import run_bass_kernel_spmd

    nc = _get(1)
    in_maps = _shard_inputs(hidden_states, W1, b1, W2, b2, atom_indices)
    res = run_bass_kernel_spmd(nc, in_maps, list(range(NCORES)))
    return _unshard(res.results, atom_mask)
